# revision 39
# baseline (speedup 1.0000x reference)
"""Multi-head attention (B=4, S=2048, E=1024, H=16, D=64) on 8 TRN2 NeuronCores.

Sharding: tensor-parallel over heads -- core c computes heads 2c and 2c+1.
Each core receives the full x (cast bf16) plus its [E, 128] slices of
Wq/Wk/Wv and biases, and produces the output for feature cols 128c:128c+128
in a transposed on-device layout [B, j, d, h, sq]; the host permutes back
and concatenates along the feature dim (pure unshard/layout, no arithmetic).

Per-core dataflow (layouts chosen so nothing is ever transposed on the PE
except v, and the y-output needs no transpose at all):
  x  --DMA-transpose-->  xT [E-chunk=128, S] (bf16)
  qT = (Wq^T xT)/8 + bq/8   [128(d, 2 heads), S]   (PE + DVE psum->sbuf)
  kT =  Wk^T xT + bk        [128, S]
  vT =  Wv^T xT + bv        [128, S] --PE transpose--> v_aug [S, 228]
        (v_aug row layout per key: [v_h0(64) | 1 | v_h1(64@65) | 1@129 | 0])
  scoresT[sk, sq] = kT^T qT  (K=64 per head; the two heads' matmuls are
        row-tiled at tile positions (0,0)/(64,0) and run CONCURRENTLY on
        the PE array -- one 512-col stream pays for both heads)
  expT = Exp(scoresT + maskbias)  (ACT, bf16 out; bias folds the key mask)
  yT_aug[128, 2, sq] += v_aug^T expT  (K=128; rows 0-63 y, row 64 denom)
  normalize in the TRANSPOSED layout: recip of denom row, partition-
  broadcast via two DVE stream_shuffles, tensor_mul -- no PE transpose.

Emission is a need-driven pipeline: projection chunks (~2 matmuls) drip
into the attention i-loop so the TensorEngine stays fed; attention for a
batch starts as soon as its q(jh0)/k(jh0)/v(jh0) blocks are emitted, which
pulls the first EXP from ~54us (coarse batch bootstrap) down to ~10us.
"""

import os
import sys
import types
from collections import deque

import numpy as np
import ml_dtypes

import concourse.bass as bass
import concourse.tile as tile
from concourse import bacc, mybir
from concourse.bass_utils import run_bass_kernel_spmd
from concourse.masks import make_identity

B, S, E, H, D = 4, 2048, 1024, 16, 64
NCORES = 8
DHC = (H // NCORES) * D  # 128 feature cols per core (2 heads)
NEG = -1.0e9  # additive mask bias for masked-out keys
BF16 = mybir.dt.bfloat16
F32 = mybir.dt.float32
SK = S // 128  # 16 key tiles per batch
SQ = S // 512  # 4 query blocks per batch
PVLAG = 5  # software-pipeline lag of PV behind scores/exp

LAST_RESULTS = None  # BassKernelResults of the most recent kernel() call


def _install_trace_hook():
    """Register the axon NTFF-profile hook so BASS_TRACE=1 works."""
    try:
        import antenv

        if "antenv.axon_hooks" in sys.modules:
            return
        mod = types.ModuleType("antenv.axon_hooks")
        _hook = [None]
        mod.set_axon_ntff_profile_hook = lambda h: _hook.__setitem__(0, h)
        mod.get_axon_ntff_profile_hook = lambda: _hook[0]
        sys.modules["antenv.axon_hooks"] = mod
        antenv.axon_hooks = mod
        from trn_agent_boot.trn_boot import _ntff_profile_via_ctypes

        so = "/opt/axon/libaxon_pjrt.so"
        if os.path.exists(so):
            mod.set_axon_ntff_profile_hook(_ntff_profile_via_ctypes(so))
    except Exception:
        pass


_install_trace_hook()


class _Ctx:
    """Shared emission state for one core's program."""


def _setup(nc, tc, ctx, aps):
    s = _Ctx()
    (s.x, wq, bq, bv, s.out) = aps

    singles = ctx.enter_context(tc.tile_pool(name="singles", bufs=1))
    s.xt_pool = ctx.enter_context(tc.tile_pool(name="xt", bufs=16))
    s.qk_pool = ctx.enter_context(tc.tile_pool(name="qk", bufs=4))
    s.v_pool = ctx.enter_context(tc.tile_pool(name="v", bufs=2))
    s.vt_pool = ctx.enter_context(tc.tile_pool(name="vt", bufs=2))
    s.exp_pool = ctx.enter_context(tc.tile_pool(name="exp", bufs=8))
    s.yb_pool = ctx.enter_context(tc.tile_pool(name="yb", bufs=2))
    s.out_pool = ctx.enter_context(tc.tile_pool(name="outs", bufs=2))
    # PSUM budget (8 banks): scores 2x[128,1024]=4, PV accum 1x[128,2,512]=2,
    # projection accum + v transposes share one double-buffered ring = 2.
    # prj bufs=2 is what decouples the PE from the DVE queue: the next
    # projection group's matmuls never wait on the previous group's
    # psum-copy, so tail work on the DVE can't head-of-line-stall the PE.
    s.ps_pool = ctx.enter_context(tc.tile_pool(name="ps", bufs=2, space="PSUM"))
    s.py_pool = ctx.enter_context(tc.tile_pool(name="py", bufs=1, space="PSUM"))
    s.prj_pool = ctx.enter_context(tc.tile_pool(name="prj", bufs=2, space="PSUM"))

    # Weight/const DMAs. wq on the scalar (ACT) queue -- it is idle until the
    # first EXP; wk/wv on sync. All are HWDGE (SWDGE descriptor gen is slow).
    wcat_sb = singles.tile([128, 3, 8, 128], BF16, tag="wcat")
    nc.scalar.dma_start(out=wcat_sb[:, :, :, :], in_=wq)
    s.w_sb = {"wq": wcat_sb[:, 0], "wk": wcat_sb[:, 1], "wv": wcat_sb[:, 2]}
    consts_sb = singles.tile([128, 66], F32, tag="consts")
    nc.scalar.dma_start(out=consts_sb[:, :], in_=bq)
    s.bq_sb = consts_sb[:, 0:1]
    s.bk_sb = consts_sb[:, 1:2]
    s.maskb = consts_sb  # bias for (b, i) at column 2 + 16*b + i
    s.bv_sb = singles.tile([128, 1], F32, tag="bv")
    nc.scalar.dma_start(out=s.bv_sb[:, :], in_=bv)
    s.ident_bf = singles.tile([128, 128], BF16, tag="ident_bf")
    make_identity(nc, s.ident_bf[:, :])
    # Reciprocal-broadcast scratch: row 64 gets 1/denom, rows 0-63 get the
    # stream_shuffle broadcast. Rows 65-95 exist only because the shuffle
    # reads a full 32-partition window [64:96); init them once.
    s.rcb = singles.tile([96, 2, 512], BF16, tag="rcb_s")
    nc.vector.memset(s.rcb[64:96, :, :], 0.0)
    s.warm = singles.tile([1, 2], F32, tag="warm")
    s.singles = singles
    s.ngroups = 0
    s.proj = {}
    s.prog = {b: {"q": -1, "kv": -1} for b in range(B)}
    return s


def _warm_act(nc, s):
    """Dummy Exp so the ~2.7us ACT table load happens during the bootstrap
    DMAs instead of on the first real EXP's critical path."""
    nc.vector.memset(s.warm[:, 0:1], 0.0)
    nc.scalar.activation(
        out=s.warm[:, 1:2], in_=s.warm[:, 0:1],
        func=mybir.ActivationFunctionType.Exp, scale=1.0)


def _gen_proj(nc, s, b):
    """Generator: emits batch b's xT loads + q/k/v projections in the order
    [q0, k0, v0, k1, v1, k2, v2, k3, v3, q1, q2, q3], yielding "c" after
    every ~2 matmuls and ("q"|"kv", jh) progress markers after each group.
    """
    mult, add = mybir.AluOpType.mult, mybir.AluOpType.add

    qT = s.qk_pool.tile([128, S], BF16, tag="qk", name=f"qT{b}")
    kT = s.qk_pool.tile([128, S], BF16, tag="qk", name=f"kT{b}")
    v_sb = s.v_pool.tile([128, SK, 228], BF16, tag="v", name=f"v{b}")
    s.proj[b] = (qT, kT, v_sb)

    # Full-chunk DMA-transposes, all on the sync queue. A transpose occupies
    # its trigger queue for the whole transfer (~2.5us per 512KB chunk), so
    # full chunks halve the serial queue time vs smaller pieces; the other
    # HWDGE queue (scalar) proved racy for transposes on HW. Batch 0's
    # projection matmuls pipeline chunk-by-chunk behind this stream.
    xt = [s.xt_pool.tile([128, S], BF16, tag="xt", name=f"xt{b}_{c}")
          for c in range(8)]
    if b == 0:
        # Halves, first halves first: the first projection groups can start
        # after half of the batch-0 load instead of all of it.
        for lo, hi in ((0, 1024), (1024, 2048)):
            for c in range(8):
                nc.sync.dma_start_transpose(
                    out=xt[c][:, lo:hi],
                    in_=s.x[b, lo:hi, 128 * c:128 * (c + 1)])
    else:
        for c in range(8):
            nc.sync.dma_start_transpose(
                out=xt[c][:, :], in_=s.x[b, :, 128 * c:128 * (c + 1)])

    def xt_at(c, jh):
        return xt[c][:, 512 * jh:512 * (jh + 1)]
    nc.vector.memset(v_sb[:, :, 64:65], 1.0)
    nc.vector.memset(v_sb[:, :, 129:130], 1.0)
    nc.vector.memset(v_sb[:, :, 130:228], 0.0)
    yield "c"

    def accum_tile(name):
        return s.prj_pool.tile([128, 512], F32, tag="prj", name=name)

    def q_or_k(name, dest, bias_sb, scale, jh):
        w = s.w_sb[name]
        ps = accum_tile("pj")
        for c in range(8):
            nc.tensor.matmul(
                ps[:, :], w[:, c, :], xt_at(c, jh),
                start=(c == 0), stop=(c == 7))
            if c % 2 == 1:
                yield "c"
        nc.vector.tensor_scalar(
            out=dest[:, 512 * jh:512 * (jh + 1)], in0=ps[:, :],
            scalar1=scale, scalar2=bias_sb[:, :], op0=mult, op1=add)
        yield "c"

    def v_proj(jh):
        # v: project to vT (+bv), then PE-transpose back to natural [s, d]
        # layout with fused ones-columns (softmax denominator) per head.
        w = s.w_sb["wv"]
        ps = accum_tile("pv")
        for c in range(8):
            nc.tensor.matmul(
                ps[:, :], w[:, c, :], xt_at(c, jh),
                start=(c == 0), stop=(c == 7))
            if c % 2 == 1:
                yield "c"
        vt = s.vt_pool.tile([128, 512], BF16, tag="vt", name="vt")
        # On ACT (not DVE): the PE's v-transposes consume vt, and the DVE
        # queue can carry multi-us tail blobs that would delay it.
        nc.scalar.activation(
            out=vt[:, :], in_=ps[:, :],
            func=mybir.ActivationFunctionType.Identity,
            bias=s.bv_sb[:, :], scale=1.0)
        yield "c"
        for t in range(4):
            i = 4 * jh + t
            pv = s.prj_pool.tile([128, 128], BF16, tag="prj", name="pvt")
            nc.tensor.transpose(
                pv[:, :], vt[:, 128 * t:128 * (t + 1)], s.ident_bf[:, :])
            nc.vector.tensor_copy(out=v_sb[:, i, 0:64], in_=pv[:, 0:64])
            nc.vector.tensor_copy(out=v_sb[:, i, 65:129], in_=pv[:, 64:128])
            yield "c"

    yield from q_or_k("wq", qT, s.bq_sb, 0.125, 0)
    yield ("q", 0)
    for jh in range(4):
        yield from q_or_k("wk", kT, s.bk_sb, 1.0, jh)
        yield from v_proj(jh)
        yield ("kv", jh)
    for jh in (1, 2, 3):
        yield from q_or_k("wq", qT, s.bq_sb, 0.125, jh)
        yield ("q", jh)


def _pv(nc, py, v_sb, ex, i):
    for h in range(2):
        nc.tensor.matmul(
            py[:, h, :], v_sb[:, i, 65 * h:65 * h + 128],
            ex[:, 512 * h:512 * (h + 1)],
            start=(i == 0), stop=(i == SK - 1))


def _gen_tail(nc, s, b, j, py):
    """Normalize in the transposed layout and store, as a generator that the
    master drips into the next block's i-loop so the single-lane reciprocal
    chunks interleave with projection psum-copies on the DVE queue.

    Structured per-head so the dependency chain pipelines: while head 0's
    multiply runs on GPSIMD, head 1's reciprocal runs on the DVE. This
    matters most for the very last block, whose chain is pure tail latency.
    """
    # Copy the 65 useful rows out of PSUM first so py's banks free up after
    # ~0.9us instead of at the end of the whole normalize chain.
    yb = s.yb_pool.tile([65, 2, 512], F32, tag="yb", name="yb")
    nc.vector.tensor_copy(out=yb[:, :, :], in_=py[0:65, :, :])
    yield
    rcb = s.rcb
    bmask = [0] * 32  # every output lane reads lane 0 (= partition 64)
    ob = s.out_pool.tile([64, 2, 512], F32, tag="outs", name="ob")
    for h in range(2):
        # 1/denom with the (HW-proven) DVE reciprocal. The row lives on one
        # partition (one DVE lane), so it costs ~6.4ns/elem; split into
        # chunks so it doesn't head-of-line-block the DVE queue.
        for m in range(2):
            with nc.allow_low_precision(reason="bf16 recip-denom, ~0.2% err"):
                nc.vector.reciprocal(
                    rcb[64:65, h, 256 * m:256 * (m + 1)],
                    yb[64:65, h, 256 * m:256 * (m + 1)])
            yield
        nc.vector.stream_shuffle(
            rcb[0:32, h, :], rcb[64:96, h, :], mask=bmask)
        yield
        nc.vector.stream_shuffle(
            rcb[32:64, h, :], rcb[64:96, h, :], mask=bmask)
        yield
        # The final multiply runs on the (idle) GPSIMD engine so the DVE
        # queue stays clear for projection psum-copies.
        nc.gpsimd.tensor_mul(ob[:, h, :], yb[0:64, h, :], rcb[0:64, h, :])
        yield
        nc.gpsimd.dma_start(out=s.out[b, j, :, h], in_=ob[:, h, :])


def _emit_body(nc, tc, ctx, aps):
    s = _setup(nc, tc, ctx, aps)
    _warm_act(nc, s)

    gens = deque((b, _gen_proj(nc, s, b)) for b in range(B))

    def drain(n=1):
        for _ in range(n):
            if not gens:
                return
            gb, g = gens[0]
            tok = next(g, None)
            if tok is None:
                gens.popleft()
            elif isinstance(tok, tuple):
                s.prog[gb][tok[0]] = tok[1]

    def drain_until(b, key, jh):
        while s.prog[b][key] < jh:
            assert gens, f"proj gen exhausted before {b=} {key=} {jh=}"
            drain(1)

    drain_until(0, "kv", 0)  # bootstrap: q0/k0/v0 of batch 0

    DONE = object()
    tail = None
    for b in range(B):
        for j in range(SQ):
            drain_until(b, "q", j)
            qT, kT, v_sb = s.proj[b]
            py = s.py_pool.tile([128, 2, 512], F32, tag="py", name="py")
            exs = {}
            for i in range(SK):
                if j == 0:
                    drain_until(b, "kv", i // 4)
                ps = s.ps_pool.tile([128, 1024], F32, tag="ps", name="psc")
                for h in range(2):
                    hp = slice(64 * h, 64 * (h + 1))
                    nc.tensor.matmul(
                        ps[:, 512 * h:512 * (h + 1)],
                        kT[hp, 128 * i:128 * (i + 1)],
                        qT[hp, 512 * j:512 * (j + 1)],
                        start=True, stop=True)
                ex = s.exp_pool.tile([128, 1024], BF16, tag="exp", name="ex")
                nc.scalar.activation(
                    out=ex[:, :], in_=ps[:, :],
                    func=mybir.ActivationFunctionType.Exp,
                    bias=s.maskb[:, 2 + 16 * b + i:3 + 16 * b + i], scale=1.0)
                exs[i] = ex
                if i >= PVLAG:
                    _pv(nc, py, v_sb, exs.pop(i - PVLAG), i - PVLAG)
                # Drip the previous block's tail starting at slot 2: at the
                # block boundary the DVE queue must first serve the in-flight
                # projection group's psum-copy (prj is single-buffered), or
                # the PE stalls behind it.
                if tail is not None and i >= 2 and next(tail, DONE) is DONE:
                    tail = None
                # No background drains in the first tiles of batch 0: a
                # drained proj matmul waiting on not-yet-landed xT data would
                # head-of-line-block the (FIFO) PE queue behind it.
                if b == 0 and j == 0 and i < 3:
                    continue
                drain(2 if b == 0 else 1)
            for ii in range(SK - PVLAG, SK):
                _pv(nc, py, v_sb, exs.pop(ii), ii)
            while tail is not None:  # should be drained already
                if next(tail, DONE) is DONE:
                    tail = None
            tail = _gen_tail(nc, s, b, j, py)
    while tail is not None:
        if next(tail, DONE) is DONE:
            tail = None
    while gens:
        drain(1)


def _build():
    from contextlib import ExitStack

    nc = bacc.Bacc("TRN2", target_bir_lowering=False, debug=False)
    x = nc.dram_tensor("x", [B, S, E], BF16, kind="ExternalInput").ap()
    wq = nc.dram_tensor("wcat", [128, 3, 8, 128], BF16,
                        kind="ExternalInput").ap()
    bq = nc.dram_tensor("consts", [128, 66], F32, kind="ExternalInput").ap()
    bv = nc.dram_tensor("bv", [128, 1], F32, kind="ExternalInput").ap()
    out = nc.dram_tensor("out", [B, SQ, 64, 2, 512], F32,
                         kind="ExternalOutput").ap()
    aps = (x, wq, bq, bv, out)
    with tile.TileContext(nc) as tc:
        with ExitStack() as ctx:
            _emit_body(nc, tc, ctx, aps)
    nc.compile()
    return nc


_BUILD_CACHE = {}


def _get_built():
    if "nc" not in _BUILD_CACHE:
        _BUILD_CACHE["nc"] = _build()
    return _BUILD_CACHE["nc"]


def kernel(x, mask, Wq, bq, Wk, bk, Wv, bv):
    global LAST_RESULTS
    bf16 = ml_dtypes.bfloat16
    x_bf = np.asarray(x, dtype=np.float32).astype(bf16)
    mask_f = np.asarray(mask).astype(np.float32)
    maskb = (mask_f - 1.0) * (-NEG)  # 0 where mask==1, NEG where mask==0
    maskb = np.ascontiguousarray(
        maskb.reshape(B, S // 128, 128).transpose(2, 0, 1)).astype(np.float32)

    nc = _get_built()

    in_maps = []
    for c in range(NCORES):
        sl = slice(DHC * c, DHC * (c + 1))

        def warr(w):
            w = np.asarray(w, dtype=np.float32)[:, sl].astype(bf16)
            return np.ascontiguousarray(
                w.reshape(8, 128, 128).transpose(1, 0, 2))

        wcat = np.stack([warr(Wq), warr(Wk), warr(Wv)], axis=1)
        consts = np.empty((128, 66), dtype=np.float32)
        consts[:, 0] = np.asarray(bq, dtype=np.float32)[sl] / 8.0
        consts[:, 1] = np.asarray(bk, dtype=np.float32)[sl]
        consts[:, 2:66] = maskb.reshape(128, 64)
        in_maps.append({
            "x": x_bf,
            "wcat": np.ascontiguousarray(wcat),
            "consts": consts,
            "bv": np.ascontiguousarray(
                np.asarray(bv, dtype=np.float32)[sl].reshape(128, 1)),
        })

    res = run_bass_kernel_spmd(nc, in_maps, core_ids=list(range(NCORES)))
    LAST_RESULTS = res
    # Device output is [B, j, d, h, sq]; permute to [B, S, 128] per core
    # (pure unshard/layout fixup on host) and concat cores on features.
    outs = []
    for c in range(NCORES):
        dev = res.results[c]["out"]  # (B, SQ, 64, 2, 512)
        outs.append(np.transpose(dev, (0, 1, 4, 3, 2)).reshape(B, S, DHC))
    return np.concatenate(outs, axis=-1)


# revision 40
# speedup vs baseline: 1.0304x; 1.0304x over previous
"""Multi-head attention (B=4, S=2048, E=1024, H=16, D=64) on 8 TRN2 NeuronCores.

Sharding: tensor-parallel over heads -- core c computes heads 2c and 2c+1.
Each core receives the full x (cast bf16) plus its [E, 128] slices of
Wq/Wk/Wv and biases, and produces the output for feature cols 128c:128c+128
in a transposed on-device layout [B, j, d, h, sq]; the host permutes back
and concatenates along the feature dim (pure unshard/layout, no arithmetic).

Per-core dataflow (layouts chosen so nothing is ever transposed on the PE
except v, and the y-output needs no transpose at all):
  x  --DMA-transpose-->  xT [E-chunk=128, S] (bf16)
  qT = (Wq^T xT)/8 + bq/8   [128(d, 2 heads), S]   (PE + DVE psum->sbuf)
  kT =  Wk^T xT + bk        [128, S]
  vT =  Wv^T xT + bv        [128, S] --PE transpose--> v_aug [S, 228]
        (v_aug row layout per key: [v_h0(64) | 1 | v_h1(64@65) | 1@129 | 0])
  scoresT[sk, sq] = kT^T qT  (K=64 per head; the two heads' matmuls are
        row-tiled at tile positions (0,0)/(64,0) and run CONCURRENTLY on
        the PE array -- one 512-col stream pays for both heads)
  expT = Exp(scoresT + maskbias)  (ACT, bf16 out; bias folds the key mask)
  yT_aug[128, 2, sq] += v_aug^T expT  (K=128; rows 0-63 y, row 64 denom)
  normalize in the TRANSPOSED layout: recip of denom row, partition-
  broadcast via two DVE stream_shuffles, tensor_mul -- no PE transpose.

Emission is a need-driven pipeline: projection chunks (~2 matmuls) drip
into the attention i-loop so the TensorEngine stays fed; attention for a
batch starts as soon as its q(jh0)/k(jh0)/v(jh0) blocks are emitted, which
pulls the first EXP from ~54us (coarse batch bootstrap) down to ~10us.
"""

import os
import sys
import types
from collections import deque

import numpy as np
import ml_dtypes

import concourse.bass as bass
import concourse.tile as tile
from concourse import bacc, mybir
from concourse.bass_utils import run_bass_kernel_spmd
from concourse.masks import make_identity

B, S, E, H, D = 4, 2048, 1024, 16, 64
NCORES = 8
DHC = (H // NCORES) * D  # 128 feature cols per core (2 heads)
NEG = -1.0e9  # additive mask bias for masked-out keys
BF16 = mybir.dt.bfloat16
F32 = mybir.dt.float32
SK = S // 128  # 16 key tiles per batch
SQ = S // 512  # 4 query blocks per batch
PVLAG = 5  # software-pipeline lag of PV behind scores/exp

LAST_RESULTS = None  # BassKernelResults of the most recent kernel() call


def _install_trace_hook():
    """Register the axon NTFF-profile hook so BASS_TRACE=1 works."""
    try:
        import antenv

        if "antenv.axon_hooks" in sys.modules:
            return
        mod = types.ModuleType("antenv.axon_hooks")
        _hook = [None]
        mod.set_axon_ntff_profile_hook = lambda h: _hook.__setitem__(0, h)
        mod.get_axon_ntff_profile_hook = lambda: _hook[0]
        sys.modules["antenv.axon_hooks"] = mod
        antenv.axon_hooks = mod
        from trn_agent_boot.trn_boot import _ntff_profile_via_ctypes

        so = "/opt/axon/libaxon_pjrt.so"
        if os.path.exists(so):
            mod.set_axon_ntff_profile_hook(_ntff_profile_via_ctypes(so))
    except Exception:
        pass


_install_trace_hook()


class _Ctx:
    """Shared emission state for one core's program."""


def _setup(nc, tc, ctx, aps):
    s = _Ctx()
    (s.x, wq, bq, bv, s.out) = aps

    singles = ctx.enter_context(tc.tile_pool(name="singles", bufs=1))
    s.xt_pool = ctx.enter_context(tc.tile_pool(name="xt", bufs=16))
    s.qk_pool = ctx.enter_context(tc.tile_pool(name="qk", bufs=4))
    s.v_pool = ctx.enter_context(tc.tile_pool(name="v", bufs=2))
    s.vt_pool = ctx.enter_context(tc.tile_pool(name="vt", bufs=2))
    s.exp_pool = ctx.enter_context(tc.tile_pool(name="exp", bufs=8))
    s.yb_pool = ctx.enter_context(tc.tile_pool(name="yb", bufs=2))
    s.out_pool = ctx.enter_context(tc.tile_pool(name="outs", bufs=2))
    # PSUM budget (8 banks): scores 2x[128,1024]=4, PV accum 1x[128,2,512]=2,
    # projection accum + v transposes share one double-buffered ring = 2.
    # prj bufs=2 is what decouples the PE from the DVE queue: the next
    # projection group's matmuls never wait on the previous group's
    # psum-copy, so tail work on the DVE can't head-of-line-stall the PE.
    s.ps_pool = ctx.enter_context(tc.tile_pool(name="ps", bufs=2, space="PSUM"))
    s.py_pool = ctx.enter_context(tc.tile_pool(name="py", bufs=1, space="PSUM"))
    s.prj_pool = ctx.enter_context(tc.tile_pool(name="prj", bufs=2, space="PSUM"))

    # Weight/const DMAs. wq on the scalar (ACT) queue -- it is idle until the
    # first EXP; wk/wv on sync. All are HWDGE (SWDGE descriptor gen is slow).
    wcat_sb = singles.tile([128, 3, 8, 128], BF16, tag="wcat")
    nc.scalar.dma_start(out=wcat_sb[:, :, :, :], in_=wq)
    s.w_sb = {"wq": wcat_sb[:, 0], "wk": wcat_sb[:, 1], "wv": wcat_sb[:, 2]}
    consts_sb = singles.tile([128, 66], F32, tag="consts")
    nc.scalar.dma_start(out=consts_sb[:, :], in_=bq)
    s.bq_sb = consts_sb[:, 0:1]
    s.bk_sb = consts_sb[:, 1:2]
    s.maskb = consts_sb  # bias for (b, i) at column 2 + 16*b + i
    s.bv_sb = singles.tile([128, 1], F32, tag="bv")
    nc.scalar.dma_start(out=s.bv_sb[:, :], in_=bv)
    s.ident_bf = singles.tile([128, 128], BF16, tag="ident_bf")
    make_identity(nc, s.ident_bf[:, :])
    # Reciprocal-broadcast scratch: row 64 gets 1/denom, rows 0-63 get the
    # stream_shuffle broadcast. Rows 65-95 exist only because the shuffle
    # reads a full 32-partition window [64:96); init them once.
    s.rcb = singles.tile([96, 2, 512], BF16, tag="rcb_s")
    nc.vector.memset(s.rcb[64:96, :, :], 0.0)
    s.warm = singles.tile([1, 2], F32, tag="warm")
    s.singles = singles
    s.ngroups = 0
    s.proj = {}
    s.prog = {b: {"q": -1, "kv": -1} for b in range(B)}
    return s


def _warm_act(nc, s):
    """Dummy Exp so the ~2.7us ACT table load happens during the bootstrap
    DMAs instead of on the first real EXP's critical path."""
    nc.vector.memset(s.warm[:, 0:1], 0.0)
    nc.scalar.activation(
        out=s.warm[:, 1:2], in_=s.warm[:, 0:1],
        func=mybir.ActivationFunctionType.Exp, scale=1.0)


def _gen_proj(nc, s, b):
    """Generator: emits batch b's xT loads + q/k/v projections in the order
    [q0, k0, v0, k1, v1, k2, v2, k3, v3, q1, q2, q3], yielding "c" after
    every ~2 matmuls and ("q"|"kv", jh) progress markers after each group.
    """
    mult, add = mybir.AluOpType.mult, mybir.AluOpType.add

    qT = s.qk_pool.tile([128, S], BF16, tag="qk", name=f"qT{b}")
    kT = s.qk_pool.tile([128, S], BF16, tag="qk", name=f"kT{b}")
    v_sb = s.v_pool.tile([128, SK, 228], BF16, tag="v", name=f"v{b}")
    s.proj[b] = (qT, kT, v_sb)

    # Full-chunk DMA-transposes, all on the sync queue. A transpose occupies
    # its trigger queue for the whole transfer (~2.5us per 512KB chunk), so
    # full chunks halve the serial queue time vs smaller pieces; the other
    # HWDGE queue (scalar) proved racy for transposes on HW. Batch 0's
    # projection matmuls pipeline chunk-by-chunk behind this stream.
    xt = [s.xt_pool.tile([128, S], BF16, tag="xt", name=f"xt{b}_{c}")
          for c in range(8)]
    if b == 0:
        # Halves, first halves first: the first projection groups can start
        # after half of the batch-0 load instead of all of it.
        for lo, hi in ((0, 1024), (1024, 2048)):
            for c in range(8):
                nc.sync.dma_start_transpose(
                    out=xt[c][:, lo:hi],
                    in_=s.x[b, lo:hi, 128 * c:128 * (c + 1)])
    else:
        for c in range(8):
            nc.sync.dma_start_transpose(
                out=xt[c][:, :], in_=s.x[b, :, 128 * c:128 * (c + 1)])

    def xt_at(c, jh):
        return xt[c][:, 512 * jh:512 * (jh + 1)]
    nc.vector.memset(v_sb[:, :, 64:65], 1.0)
    nc.vector.memset(v_sb[:, :, 129:130], 1.0)
    nc.vector.memset(v_sb[:, :, 130:228], 0.0)
    yield "c"

    def accum_tile(name):
        return s.prj_pool.tile([128, 512], F32, tag="prj", name=name)

    def q_or_k(name, dest, bias_sb, scale, jh):
        w = s.w_sb[name]
        ps = accum_tile("pj")
        for c in range(8):
            nc.tensor.matmul(
                ps[:, :], w[:, c, :], xt_at(c, jh),
                start=(c == 0), stop=(c == 7))
            if c % 2 == 1:
                yield "c"
        nc.vector.tensor_scalar(
            out=dest[:, 512 * jh:512 * (jh + 1)], in0=ps[:, :],
            scalar1=scale, scalar2=bias_sb[:, :], op0=mult, op1=add)
        yield "c"

    def v_proj(jh):
        # v: project to vT (+bv), then PE-transpose back to natural [s, d]
        # layout with fused ones-columns (softmax denominator) per head.
        w = s.w_sb["wv"]
        ps = accum_tile("pv")
        for c in range(8):
            nc.tensor.matmul(
                ps[:, :], w[:, c, :], xt_at(c, jh),
                start=(c == 0), stop=(c == 7))
            if c % 2 == 1:
                yield "c"
        vt = s.vt_pool.tile([128, 512], BF16, tag="vt", name="vt")
        # On ACT (not DVE): the PE's v-transposes consume vt, and the DVE
        # queue can carry multi-us tail blobs that would delay it.
        nc.scalar.activation(
            out=vt[:, :], in_=ps[:, :],
            func=mybir.ActivationFunctionType.Identity,
            bias=s.bv_sb[:, :], scale=1.0)
        yield "c"
        for t in range(4):
            i = 4 * jh + t
            pv = s.prj_pool.tile([128, 128], BF16, tag="prj", name="pvt")
            nc.tensor.transpose(
                pv[:, :], vt[:, 128 * t:128 * (t + 1)], s.ident_bf[:, :])
            nc.vector.tensor_copy(out=v_sb[:, i, 0:64], in_=pv[:, 0:64])
            nc.vector.tensor_copy(out=v_sb[:, i, 65:129], in_=pv[:, 64:128])
            yield "c"

    yield from q_or_k("wq", qT, s.bq_sb, 0.125, 0)
    yield ("q", 0)
    for jh in range(4):
        yield from q_or_k("wk", kT, s.bk_sb, 1.0, jh)
        yield from v_proj(jh)
        yield ("kv", jh)
    for jh in (1, 2, 3):
        yield from q_or_k("wq", qT, s.bq_sb, 0.125, jh)
        yield ("q", jh)


def _pv(nc, py, v_sb, ex, i):
    for h in range(2):
        nc.tensor.matmul(
            py[:, h, :], v_sb[:, i, 65 * h:65 * h + 128],
            ex[:, 512 * h:512 * (h + 1)],
            start=(i == 0), stop=(i == SK - 1))


def _gen_tail(nc, s, b, j, py):
    """Normalize in the transposed layout and store, as a generator that the
    master drips into the next block's i-loop so the single-lane reciprocal
    chunks interleave with projection psum-copies on the DVE queue.

    Structured per-head so the dependency chain pipelines: while head 0's
    multiply runs on GPSIMD, head 1's reciprocal runs on the DVE. This
    matters most for the very last block, whose chain is pure tail latency.
    """
    # Copy the 65 useful rows out of PSUM first so py's banks free up after
    # ~0.9us instead of at the end of the whole normalize chain.
    yb = s.yb_pool.tile([65, 2, 512], F32, tag="yb", name="yb")
    nc.vector.tensor_copy(out=yb[:, :, :], in_=py[0:65, :, :])
    yield
    # 1/denom with the (HW-proven) DVE reciprocal. The row lives on one
    # partition (one DVE lane), so it costs ~6.4ns/elem; split into 4
    # chunks so it doesn't head-of-line-block the DVE queue.
    rcb = s.rcb
    for h in range(2):
        for m in range(2):
            with nc.allow_low_precision(reason="bf16 recip-denom, ~0.2% err"):
                nc.vector.reciprocal(
                    rcb[64:65, h, 256 * m:256 * (m + 1)],
                    yb[64:65, h, 256 * m:256 * (m + 1)])
            yield
    bmask = [0] * 32  # every output lane reads lane 0 (= partition 64)
    nc.vector.stream_shuffle(rcb[0:32, :, :], rcb[64:96, :, :], mask=bmask)
    yield
    nc.vector.stream_shuffle(rcb[32:64, :, :], rcb[64:96, :, :], mask=bmask)
    yield
    # The final multiplies run on the (idle) GPSIMD engine so the DVE queue
    # stays clear for projection psum-copies.
    ob = s.out_pool.tile([64, 2, 512], F32, tag="outs", name="ob")
    for h in range(2):
        nc.gpsimd.tensor_mul(ob[:, h, :], yb[0:64, h, :], rcb[0:64, h, :])
        yield
    nc.gpsimd.dma_start(out=s.out[b, j], in_=ob[:, :, :])


def _emit_body(nc, tc, ctx, aps):
    s = _setup(nc, tc, ctx, aps)
    _warm_act(nc, s)

    gens = deque((b, _gen_proj(nc, s, b)) for b in range(B))

    def drain(n=1):
        for _ in range(n):
            if not gens:
                return
            gb, g = gens[0]
            tok = next(g, None)
            if tok is None:
                gens.popleft()
            elif isinstance(tok, tuple):
                s.prog[gb][tok[0]] = tok[1]

    def drain_until(b, key, jh):
        while s.prog[b][key] < jh:
            assert gens, f"proj gen exhausted before {b=} {key=} {jh=}"
            drain(1)

    drain_until(0, "kv", 0)  # bootstrap: q0/k0/v0 of batch 0

    DONE = object()
    tail = None
    for b in range(B):
        for j in range(SQ):
            drain_until(b, "q", j)
            qT, kT, v_sb = s.proj[b]
            py = s.py_pool.tile([128, 2, 512], F32, tag="py", name="py")
            exs = {}
            for i in range(SK):
                if j == 0:
                    drain_until(b, "kv", i // 4)
                ps = s.ps_pool.tile([128, 1024], F32, tag="ps", name="psc")
                for h in range(2):
                    hp = slice(64 * h, 64 * (h + 1))
                    nc.tensor.matmul(
                        ps[:, 512 * h:512 * (h + 1)],
                        kT[hp, 128 * i:128 * (i + 1)],
                        qT[hp, 512 * j:512 * (j + 1)],
                        start=True, stop=True)
                ex = s.exp_pool.tile([128, 1024], BF16, tag="exp", name="ex")
                nc.scalar.activation(
                    out=ex[:, :], in_=ps[:, :],
                    func=mybir.ActivationFunctionType.Exp,
                    bias=s.maskb[:, 2 + 16 * b + i:3 + 16 * b + i], scale=1.0)
                exs[i] = ex
                if i >= PVLAG:
                    _pv(nc, py, v_sb, exs.pop(i - PVLAG), i - PVLAG)
                # Drip the previous block's tail starting at slot 2: at the
                # block boundary the DVE queue must first serve the in-flight
                # projection group's psum-copy (prj is single-buffered), or
                # the PE stalls behind it.
                if tail is not None and i >= 2 and next(tail, DONE) is DONE:
                    tail = None
                # No background drains in the first tiles of batch 0: a
                # drained proj matmul waiting on not-yet-landed xT data would
                # head-of-line-block the (FIFO) PE queue behind it.
                if b == 0 and j == 0 and i < 3:
                    continue
                drain(2 if b == 0 else 1)
            for ii in range(SK - PVLAG, SK):
                _pv(nc, py, v_sb, exs.pop(ii), ii)
            while tail is not None:  # should be drained already
                if next(tail, DONE) is DONE:
                    tail = None
            tail = _gen_tail(nc, s, b, j, py)
    while tail is not None:
        if next(tail, DONE) is DONE:
            tail = None
    while gens:
        drain(1)


def _build():
    from contextlib import ExitStack

    nc = bacc.Bacc("TRN2", target_bir_lowering=False, debug=False)
    x = nc.dram_tensor("x", [B, S, E], BF16, kind="ExternalInput").ap()
    wq = nc.dram_tensor("wcat", [128, 3, 8, 128], BF16,
                        kind="ExternalInput").ap()
    bq = nc.dram_tensor("consts", [128, 66], F32, kind="ExternalInput").ap()
    bv = nc.dram_tensor("bv", [128, 1], F32, kind="ExternalInput").ap()
    out = nc.dram_tensor("out", [B, SQ, 64, 2, 512], F32,
                         kind="ExternalOutput").ap()
    aps = (x, wq, bq, bv, out)
    with tile.TileContext(nc) as tc:
        with ExitStack() as ctx:
            _emit_body(nc, tc, ctx, aps)
    nc.compile()
    return nc


_BUILD_CACHE = {}


def _get_built():
    if "nc" not in _BUILD_CACHE:
        _BUILD_CACHE["nc"] = _build()
    return _BUILD_CACHE["nc"]


def kernel(x, mask, Wq, bq, Wk, bk, Wv, bv):
    global LAST_RESULTS
    bf16 = ml_dtypes.bfloat16
    x_bf = np.asarray(x, dtype=np.float32).astype(bf16)
    mask_f = np.asarray(mask).astype(np.float32)
    maskb = (mask_f - 1.0) * (-NEG)  # 0 where mask==1, NEG where mask==0
    maskb = np.ascontiguousarray(
        maskb.reshape(B, S // 128, 128).transpose(2, 0, 1)).astype(np.float32)

    nc = _get_built()

    in_maps = []
    for c in range(NCORES):
        sl = slice(DHC * c, DHC * (c + 1))

        def warr(w):
            w = np.asarray(w, dtype=np.float32)[:, sl].astype(bf16)
            return np.ascontiguousarray(
                w.reshape(8, 128, 128).transpose(1, 0, 2))

        wcat = np.stack([warr(Wq), warr(Wk), warr(Wv)], axis=1)
        consts = np.empty((128, 66), dtype=np.float32)
        consts[:, 0] = np.asarray(bq, dtype=np.float32)[sl] / 8.0
        consts[:, 1] = np.asarray(bk, dtype=np.float32)[sl]
        consts[:, 2:66] = maskb.reshape(128, 64)
        in_maps.append({
            "x": x_bf,
            "wcat": np.ascontiguousarray(wcat),
            "consts": consts,
            "bv": np.ascontiguousarray(
                np.asarray(bv, dtype=np.float32)[sl].reshape(128, 1)),
        })

    res = run_bass_kernel_spmd(nc, in_maps, core_ids=list(range(NCORES)))
    LAST_RESULTS = res
    # Device output is [B, j, d, h, sq]; permute to [B, S, 128] per core
    # (pure unshard/layout fixup on host) and concat cores on features.
    outs = []
    for c in range(NCORES):
        dev = res.results[c]["out"]  # (B, SQ, 64, 2, 512)
        outs.append(np.transpose(dev, (0, 1, 4, 3, 2)).reshape(B, S, DHC))
    return np.concatenate(outs, axis=-1)
